# revision 54
# baseline (speedup 1.0000x reference)
"""Causal multi-head attention on 8 Trainium2 NeuronCores (Bass/Tile).

Problem (hardcoded): x[2,2048,1024], W_qkv[1024,3072], b_qkv[3072],
W_proj[1024,1024], b_proj[1024]; 16 heads, head_dim 64, causal softmax.

Sharding: tensor-parallel over heads — core c owns heads (2c, 2c+1).
Each core computes qkv for its 2 heads (needs full x), the causal
attention for those heads, and a row-parallel partial of the output
projection. Host sums the 8 partials and adds the (precomputable) bias
terms.

Device layout choices (all chosen to avoid on-device transposes):
  - x is passed host-transposed as xT[1024, 4096] so the PE (which
    contracts over the partition dim) can consume it directly.
  - q,k are produced transposed (qT/kT [128=2*64, 4096]) straight out of
    the qkv matmul; v is produced in natural [token, feat] layout by
    swapping the matmul operands.
  - attention scores are computed as S^T = k @ q^T in [tk, tq] blocks of
    [128, 2*512] (both heads side by side, one exp per block); causal
    masking is a 0/1 multiply on VectorE for diagonal blocks only;
    fully-masked blocks are skipped.
  - P@V uses V widened with a 64-column ones block, so the softmax
    denominator lands bank-aligned under the numerator in the same PSUM
    tile; normalization is an elementwise reciprocal+multiply.
  - emission is software-pipelined (A(n) / B(b,i) / C slices
    interleaved) so the PE-heavy qkv/proj phases overlap the
    ScalarE-heavy exp phase.
Matmuls run in float32r (TF32-like, full PE rate; plain fp32 is 4x
slower); accumulation is fp32 in PSUM.
"""

import numpy as np

import concourse.bass as bass
import concourse.tile as tile
from concourse import bacc, mybir
from concourse.bass_utils import run_bass_kernel_spmd

B, T, C = 2, 2048, 1024
H, D = 16, 64
TOK = B * T            # 4096
P = 128
NQ = 512               # q-chunk (moving free dim per head)
KB = 128               # k-block (PSUM partition dim)
KO = C // P            # 8 contraction subtiles
NCHUNK = TOK // NQ     # 8 token chunks
QC = T // NQ           # 4 q-chunks per batch
KBB = T // KB          # 16 k-blocks per batch
F32 = mybir.dt.float32
F32R = mybir.dt.float32r

_CACHE = {}


def _build(debug_taps=False):
    nc = bacc.Bacc("TRN2", target_bir_lowering=False, debug=False, num_devices=8)
    marks = []
    _CACHE["marks"] = marks

    def mark(lbl):
        marks.append((nc.next_id(), lbl))

    xt_d = nc.dram_tensor("xt", [C, TOK], F32R, kind="ExternalInput").ap()
    wqk_d = nc.dram_tensor("wqk", [C, 256], F32R, kind="ExternalInput").ap()
    bqk_d = nc.dram_tensor("bqk", [P, 2], F32, kind="ExternalInput").ap()
    wv_d = nc.dram_tensor("wv", [C, P], F32R, kind="ExternalInput").ap()
    wproj_d = nc.dram_tensor("wproj", [P, C], F32R, kind="ExternalInput").ap()
    masks_d = nc.dram_tensor("masks", [P, P], F32R, kind="ExternalInput").ap()
    ident_d = nc.dram_tensor("ident", [P, P], F32R, kind="ExternalInput").ap()
    y_d = nc.dram_tensor("y", [TOK, C], F32, kind="ExternalOutput").ap()
    dbg = {}
    if debug_taps:
        dbg["qT"] = nc.dram_tensor("dbg_qT", [P, TOK], F32, kind="ExternalOutput").ap()
        dbg["kT"] = nc.dram_tensor("dbg_kT", [P, TOK], F32, kind="ExternalOutput").ap()
        dbg["v0"] = nc.dram_tensor("dbg_v0", [P, 2 * KBB, 2 * D], F32, kind="ExternalOutput").ap()
        dbg["v1"] = nc.dram_tensor("dbg_v1", [P, 2 * KBB, 2 * D], F32, kind="ExternalOutput").ap()
        dbg["attns"] = nc.dram_tensor("dbg_attns", [P, TOK], F32, kind="ExternalOutput").ap()

    with tile.TileContext(nc) as tc:
        with tc.tile_pool(name="res", bufs=1) as res, \
             tc.tile_pool(name="xt", bufs=18) as xtp, \
             tc.tile_pool(name="pt", bufs=4) as ptp, \
             tc.tile_pool(name="ystage", bufs=12) as ysp:
            # ---- resident tensors ----
            wqk_sb = res.tile([P, KO, 256], F32R, tag="wqk")
            nc.sync.dma_start(wqk_sb[:, :, 0:P],
                              wqk_d.rearrange("(ko p) m -> p ko m", p=P)[:, :, 0:P])
            bqk_sb = res.tile([P, 2], F32, tag="bqk")
            nc.sync.dma_start(bqk_sb[:], bqk_d[:])
            wv_sb = res.tile([P, KO, P], F32R, tag="wv")
            wproj_sb = res.tile([P, C], F32R, tag="wproj")
            masks_sb = res.tile([P, P], F32R, tag="masks")
            ident_sb = res.tile([P, P], F32R, tag="ident")

            qT_sb = res.tile([P, TOK], F32R, tag="qT")
            kT_sb = res.tile([P, TOK], F32R, tag="kT")
            v_sb = [res.tile([P, 2 * KBB, 2 * D], F32R, tag=f"v{h}", name=f"v{h}")
                    for h in range(2)]
            attns_sb = res.tile([P, TOK], F32R, tag="attns")

            # ones block of v (cols D..2D-1): denominator replicator.
            # memset f32 staging then DVE copy so the f32r output is "rounded".
            ones_f32 = res.tile([P, 2 * KBB, D], F32, tag="ones_f32")
            nc.vector.memset(ones_f32[:], 1.0)
            for h in range(2):
                nc.vector.tensor_copy(v_sb[h][:, :, D:2 * D], ones_f32[:])

            # ---- filler machinery: A(qkv) and C(proj) work is split into
            # small PE quanta pumped between attention j-steps, so the PE
            # (in-order queue) always has ready work while ScalarE runs exp.
            # psF: 2 shared PSUM banks for all filler quanta.
            from collections import deque
            fill_q = deque()          # deque of (kind, closure)
            a_left_box = [0]

            def pump(k=1, max_c=None):
                n = 0
                n_c = 0
                while n < k and fill_q:
                    kind = fill_q[0][0]
                    if kind == "C" and max_c is not None and n_c >= max_c:
                        break
                    kind, f = fill_q.popleft()
                    if kind == "A":
                        a_left_box[0] -= 1
                    else:
                        n_c += 1
                    f()
                    n += 1

            def pump_all():
                pump(len(fill_q))

            def make_A_quanta(n, psF):
                st = {}

                def q_dma():
                    xts = []
                    for k in range(KO):
                        xt = xtp.tile([P, NQ], F32R, name="xt")
                        nc.sync.dma_start(
                            xt[:], xt_d[k * P:(k + 1) * P, n * NQ:(n + 1) * NQ])
                        xts.append(xt)
                    st["xts"] = xts

                def q_qk(m):
                    def f():
                        mark(f"A{n}.qk{m}")
                        pq = psF.tile([P, NQ], F32, tag=f"f{m}", name="pq")
                        for k in range(KO):
                            nc.tensor.matmul(
                                pq[:], wqk_sb[:, k, m * P:(m + 1) * P],
                                st["xts"][k][:], start=(k == 0), stop=(k == KO - 1))
                        dst = qT_sb if m == 0 else kT_sb
                        nc.vector.tensor_scalar_add(
                            dst[:, n * NQ:(n + 1) * NQ], pq[:],
                            bqk_sb[:, m:m + 1])
                    return f

                def q_vT():
                    # vT[feat, tok] accumulated with wv stationary (8 wide
                    # matmuls instead of 32 narrow ones), staged to SBUF
                    mark(f"A{n}.vT")
                    pvT = psF.tile([P, NQ], F32, tag="f0", name="pvT")
                    for k in range(KO):
                        nc.tensor.matmul(
                            pvT[:], wv_sb[:, k, :], st["xts"][k][:],
                            start=(k == 0), stop=(k == KO - 1))
                    vt = ysp.tile([P, NQ], F32R, tag="vt", name="vt", bufs=4)
                    nc.vector.tensor_copy(vt[:], pvT[:])
                    st["vt"] = vt

                def q_tp(m2):
                    # PE-transpose one [128,128] block of vT back to natural
                    # [token, feat] layout for the PV stationary operand
                    def f():
                        mark(f"A{n}.tp{m2}")
                        tp = psF.tile([P, P], F32R, tag=f"f{m2 % 2}", name="tp")
                        nc.tensor.transpose(
                            tp[:], st["vt"][:, m2 * P:(m2 + 1) * P], ident_sb[:])
                        for h in range(2):
                            nc.vector.tensor_copy(
                                v_sb[h][:, n * 4 + m2, 0:D],
                                tp[:, h * D:(h + 1) * D])
                    return f

                return [q_dma, q_qk(0), q_qk(1), q_vT,
                        q_tp(0), q_tp(1), q_tp(2), q_tp(3)]

            def make_C_quantum(m, psF, on_act=False):
                def f():
                    mark(f"C.m{m}")
                    for n2 in range(2):
                        py = psF.tile([P, NQ], F32, tag=f"f{n2}", name="py")
                        nc.tensor.matmul(
                            py[:], attns_sb[:, m * P:(m + 1) * P],
                            wproj_sb[:, n2 * NQ:(n2 + 1) * NQ],
                            start=True, stop=True)
                        ys = ysp.tile([P, NQ], F32, name="ys")
                        if on_act:
                            # kernel tail: ScalarE is idle (exps done) and the
                            # DVE queue is backed up with normalizations
                            nc.scalar.copy(ys[:], py[:])
                        else:
                            nc.vector.tensor_copy(ys[:], py[:])
                        nc.sync.dma_start(
                            y_d[m * P:(m + 1) * P, n2 * NQ:(n2 + 1) * NQ], ys[:])
                return f

            # ---- stage B chunk: attention for batch b, q-chunk i ----
            js_left_box = [80]  # total j-steps over all B chunks

            def emit_B(b, i):
                nq0 = b * T + i * NQ
                jmax = 4 * i + 4
                if True:
                    psS, psO = psS_g, psO_g
                    po = [psO.tile([P, NQ], F32, tag=f"o{h}", name=f"po{h}")
                          for h in range(2)]
                    s_tiles = {}

                    def emit_s(j):
                        s = psS.tile([P, 2 * NQ], F32, tag="s", name="s")
                        for h in range(2):
                            nc.tensor.matmul(
                                s[:, h * NQ:(h + 1) * NQ],
                                kT_sb[h * D:(h + 1) * D,
                                      b * T + j * KB: b * T + (j + 1) * KB],
                                qT_sb[h * D:(h + 1) * D, nq0:nq0 + NQ],
                                start=True, stop=True)
                        s_tiles[j] = s

                    emit_s(0)
                    budget0 = len(fill_q) * jmax // js_left_box[0]
                    js_left_box[0] -= jmax
                    taken = 0
                    for j in range(jmax):
                        mark(f"B{b}.{i}.j{j}")
                        if j + 1 < jmax:
                            emit_s(j + 1)
                        pt = ptp.tile([P, 2 * NQ], F32R, name="pt")
                        s = s_tiles.pop(j)
                        if j >= 4 * i:
                            # diagonal superblock: columns < dlt*KB are fully
                            # masked (zero), the triangle is one KB-wide
                            # sub-block, the rest is fully unmasked.
                            dlt = j - 4 * i
                            for h in range(2):
                                c0 = h * NQ
                                if dlt > 0:
                                    nc.vector.tensor_scalar_mul(
                                        pt[:, c0:c0 + dlt * KB],
                                        s[:, c0:c0 + dlt * KB], 0.0)
                                nc.scalar.activation(
                                    pt[:, c0 + dlt * KB:c0 + NQ],
                                    s[:, c0 + dlt * KB:c0 + NQ],
                                    mybir.ActivationFunctionType.Exp)
                                nc.vector.tensor_mul(
                                    pt[:, c0 + dlt * KB:c0 + (dlt + 1) * KB],
                                    pt[:, c0 + dlt * KB:c0 + (dlt + 1) * KB],
                                    masks_sb[:])
                        else:
                            nc.scalar.activation(
                                pt[:], s[:],
                                mybir.ActivationFunctionType.Exp)
                        want = budget0 * (j + 1) // jmax
                        if want > taken:
                            pump(want - taken)
                            taken = want
                        for h in range(2):
                            nc.tensor.matmul(
                                po[h][:], v_sb[h][:, b * KBB + j, :],
                                pt[:, h * NQ:(h + 1) * NQ],
                                start=(j == 0), stop=(j == jmax - 1))
                            if j == jmax - 1:
                                # normalize this head immediately: its recip
                                # runs on DVE while PE starts the other head
                                rc = ptp.tile([D, NQ], F32, tag="rc", name="rc")
                                nc.vector.reciprocal(rc[:], po[h][D:2 * D, :])
                                nc.vector.tensor_mul(
                                    attns_sb[h * D:(h + 1) * D, nq0:nq0 + NQ],
                                    po[h][0:D, :], rc[:])

            # ---- interleaved emission ----
            with tc.tile_pool(name="psF", bufs=1, space="PSUM") as psF, \
                 tc.tile_pool(name="psS", bufs=2, space="PSUM") as psS_g, \
                 tc.tile_pool(name="psO", bufs=1, space="PSUM") as psO_g:
                for n in range(NCHUNK):
                    for q in make_A_quanta(n, psF):
                        fill_q.append(("A", q))
                        a_left_box[0] += 1

                # prologue: chunk-0 xt DMAs first, then the rest of the
                # resident loads behind them on the queue
                pump(1)
                nc.sync.dma_start(
                    wqk_sb[:, :, P:2 * P],
                    wqk_d.rearrange("(ko p) m -> p ko m", p=P)[:, :, P:2 * P])
                nc.sync.dma_start(wv_sb[:],
                                  wv_d.rearrange("(ko p) m -> p ko m", p=P))
                nc.sync.dma_start(ident_sb[:], ident_d[:])
                pump(8)
                nc.sync.dma_start(masks_sb[:], masks_d[:])

                b1_order = [1, 2, 3, 0]  # small chunk last -> short tail
                for b in range(B):
                    order = list(range(QC)) if b == 0 else b1_order
                    for i in order:
                        # A chunks needed by this B chunk first
                        need = 8 * (NCHUNK - (b * QC + i + 1))
                        while a_left_box[0] > need:
                            pump(1)
                        emit_B(b, i)
                        if b == 0 and i == 0:
                            nc.sync.dma_start(wproj_sb[:], wproj_d[:])
                        if b == 0 and i == QC - 1:
                            for m in range(16):        # batch-0 proj ready
                                fill_q.append(("C", make_C_quantum(m, psF)))
                        if b == 1:
                            for m in range(16 + 4 * i, 20 + 4 * i):
                                fill_q.append(
                                    ("C", make_C_quantum(m, psF, on_act=True)))
                # trailing: drain with a wider psum pool
                with tc.tile_pool(name="psYt", bufs=3, space="PSUM") as psYt:
                    while fill_q:
                        kind, f = fill_q.popleft()
                        f()

            if debug_taps:
                nc.sync.dma_start(dbg["qT"][:], qT_sb[:].bitcast(F32))
                nc.sync.dma_start(dbg["kT"][:], kT_sb[:].bitcast(F32))
                nc.sync.dma_start(dbg["v0"][:], v_sb[0][:].bitcast(F32))
                nc.sync.dma_start(dbg["v1"][:], v_sb[1][:].bitcast(F32))
                nc.sync.dma_start(dbg["attns"][:], attns_sb[:].bitcast(F32))

    nc.compile()
    return nc


def _host_prep(x, W_qkv, b_qkv, W_proj, b_proj):
    x = np.ascontiguousarray(np.asarray(x, dtype=np.float32))
    W_qkv = np.asarray(W_qkv, dtype=np.float32)
    b_qkv = np.asarray(b_qkv, dtype=np.float32)
    W_proj = np.asarray(W_proj, dtype=np.float32)
    b_proj = np.asarray(b_proj, dtype=np.float32)

    xT = np.ascontiguousarray(x.reshape(TOK, C).T)  # [1024, 4096]
    scale = np.float32(1.0 / np.sqrt(D))

    masks = np.ascontiguousarray(
        np.triu(np.ones((P, P), dtype=np.float32)))  # [tk, tq]: tq >= tk
    ident = np.ascontiguousarray(np.eye(P, dtype=np.float32))

    in_maps = []
    for c in range(8):
        s0, s1 = c * P, (c + 1) * P
        wq = W_qkv[:, s0:s1] * scale
        wk = W_qkv[:, C + s0:C + s1]
        wv = W_qkv[:, 2 * C + s0:2 * C + s1]
        bq = b_qkv[s0:s1] * scale
        bk = b_qkv[C + s0:C + s1]
        in_maps.append({
            "xt": xT,
            "wqk": np.ascontiguousarray(np.concatenate([wq, wk], axis=1)),
            "bqk": np.ascontiguousarray(np.stack([bq, bk], axis=1)),
            "wv": np.ascontiguousarray(wv),
            "wproj": np.ascontiguousarray(W_proj[s0:s1, :]),
            "masks": masks,
            "ident": ident,
        })
    # constant bias terms folded on host:
    #   out_proj bias + (v-bias row) @ W_proj  (v bias passes through softmax)
    ybias = b_qkv[2 * C:3 * C] @ W_proj + b_proj  # [1024]
    return in_maps, ybias


def kernel(x, W_qkv, b_qkv, W_proj, b_proj):
    if "nc" not in _CACHE:
        _CACHE["nc"] = _build()
    nc = _CACHE["nc"]
    in_maps, ybias = _host_prep(x, W_qkv, b_qkv, W_proj, b_proj)
    try:
        res = run_bass_kernel_spmd(nc, in_maps, core_ids=list(range(8)))
    except Exception:
        # transient device errors (NRT_EXEC_UNIT_UNRECOVERABLE) heal on retry
        res = run_bass_kernel_spmd(nc, in_maps, core_ids=list(range(8)))
    y = np.zeros((TOK, C), dtype=np.float32)
    for c in range(8):
        y += res.results[c]["y"]
    y += ybias[None, :].astype(np.float32)
    return y.reshape(B, T, C)
